# revision 5
# baseline (speedup 1.0000x reference)
"""CrossAttention Trainium2 kernel.

Full inputs in, full output out. Sharding: data-parallel over batch (B=2),
tensor-parallel over heads (16 heads -> 4 groups of 4), 8 cores total.
Each core computes attention for its (batch, 4-head group) and a partial
output projection; the host sums the 4 partials per batch (Megatron-style
row-parallel Wout reduce) and adds bout.

Device-side dataflow is feature-major end to end:
  qT[d,q]  = (Wq_slice).T @ query.T      (lhsT=Wq slice, rhs=queryT)
  kT[d,c]  = (Wk_slice).T @ context.T
  v[c,d+1] = context @ Wv_slice (+ones col per head for softmax denom)
  ST[c,q]  = kT.T-slice @ qT-slice       (scores, transposed)
  PT[c,q]  = exp(SCALE * ST)             (ScalarE, scale fused)
  att[d+1,q] = v'.T @ PT                 (row 64 = sum_c PT = Z)
  att_n    = att[0:64] * broadcast(1/Z)  (DVE + gpsimd partition_broadcast)
  out[q,n] = att_n-chunks.T @ Wout-chunks  (token-major, partial over heads)
"""

import numpy as np
import ml_dtypes

import concourse.bass as bass
import concourse.tile as tile
from concourse import bacc, mybir
from concourse import bass2jax

BF16 = mybir.dt.bfloat16
F32 = mybir.dt.float32
NPBF16 = ml_dtypes.bfloat16

B, MODEL, H, D = 2, 1024, 16, 64
LQ, LC = 2048, 2048
N_CORES = 8
GROUPS = 4            # head groups = cores per batch
HPC = H // GROUPS     # heads per core
HD = HPC * D          # 256 head-dims per core
SCALE = D ** -0.5


def build_nc(lq=LQ, lc=LC):
    NT = MODEL // 128         # m-contraction chunks
    DT = HD // 128            # d-tiles (128 rows each) for qT/kT
    CT = lc // 128            # context chunks
    QE = 1024 if lq % 1024 == 0 else 512   # exp block (free dim of ST)
    NQE = lq // QE
    NS = QE // 512            # 512-sub-blocks per exp block

    nc = bacc.Bacc(trn_type="TRN2", target_bir_lowering=False, debug=False,
                   num_devices=N_CORES)

    qT_d = nc.dram_tensor("qT", [MODEL, lq], BF16, kind="ExternalInput").ap()
    cT_d = nc.dram_tensor("cT", [MODEL, lc], BF16, kind="ExternalInput").ap()
    wq_d = nc.dram_tensor("wq", [MODEL, HD], BF16, kind="ExternalInput").ap()
    wk_d = nc.dram_tensor("wk", [MODEL, HD], BF16, kind="ExternalInput").ap()
    wv_d = nc.dram_tensor("wv", [MODEL + 1, HD], BF16, kind="ExternalInput").ap()
    wo_d = nc.dram_tensor("wo", [HD, MODEL], BF16, kind="ExternalInput").ap()
    bq_d = nc.dram_tensor("bq", [HD, 1], F32, kind="ExternalInput").ap()
    bk_d = nc.dram_tensor("bk", [HD, 1], F32, kind="ExternalInput").ap()
    out_d = nc.dram_tensor("outp", [lq, MODEL], F32, kind="ExternalOutput").ap()

    with tile.TileContext(nc) as tc:
        with tc.tile_pool(name="w", bufs=1) as wp, \
             tc.tile_pool(name="acts", bufs=1) as acp, \
             tc.tile_pool(name="proj", bufs=1) as prp, \
             tc.tile_pool(name="ptp", bufs=6) as ptp, \
             tc.tile_pool(name="nrm", bufs=3) as nrm, \
             tc.tile_pool(name="osb", bufs=3) as osb:

            # ---- load weights / biases ----
            wq_sb = [wp.tile([128, HD], BF16, name=f"wq{a}", tag=f"wq{a}")
                     for a in range(NT)]
            wk_sb = [wp.tile([128, HD], BF16, name=f"wk{a}", tag=f"wk{a}")
                     for a in range(NT)]
            wv_sb = [wp.tile([128, HD], BF16, name=f"wv{a}", tag=f"wv{a}")
                     for a in range(NT)]
            wvb_sb = wp.tile([1, HD], BF16, name="wvb", tag="wvb")
            wo_sb = [wp.tile([128, MODEL], BF16, name=f"wo{t}", tag=f"wo{t}")
                     for t in range(DT)]
            bq_sb = [wp.tile([128, 1], F32, name=f"bq{t}", tag=f"bq{t}")
                     for t in range(DT)]
            bk_sb = [wp.tile([128, 1], F32, name=f"bk{t}", tag=f"bk{t}")
                     for t in range(DT)]
            for a in range(NT):
                nc.sync.dma_start(wq_sb[a][:], wq_d[a * 128:(a + 1) * 128, :])
                nc.sync.dma_start(wk_sb[a][:], wk_d[a * 128:(a + 1) * 128, :])
                nc.sync.dma_start(wv_sb[a][:], wv_d[a * 128:(a + 1) * 128, :])
            nc.sync.dma_start(wvb_sb[:], wv_d[MODEL:MODEL + 1, :])
            for t in range(DT):
                nc.sync.dma_start(wo_sb[t][:], wo_d[t * 128:(t + 1) * 128, :])
                nc.sync.dma_start(bq_sb[t][:], bq_d[t * 128:(t + 1) * 128, :])
                nc.sync.dma_start(bk_sb[t][:], bk_d[t * 128:(t + 1) * 128, :])

            # ---- load transposed activations ----
            qTi = [acp.tile([128, lq], BF16, name=f"qTi{a}", tag=f"qTi{a}")
                   for a in range(NT)]
            cTi = [acp.tile([128, lc], BF16, name=f"cTi{a}", tag=f"cTi{a}")
                   for a in range(NT)]
            for a in range(NT):
                nc.sync.dma_start(qTi[a][:], qT_d[a * 128:(a + 1) * 128, :])
                nc.sync.dma_start(cTi[a][:], cT_d[a * 128:(a + 1) * 128, :])
            ones_c = acp.tile([1, lc], BF16, name="ones_c", tag="ones_c")
            nc.vector.memset(ones_c[:], 1.0)

            # ---- projections ----
            kT_sb = [acp.tile([128, lc], BF16, name=f"kT{t}", tag=f"kT{t}")
                     for t in range(DT)]
            qT_sb = [acp.tile([128, lq], BF16, name=f"qT{t}", tag=f"qT{t}")
                     for t in range(DT)]
            # v: per context-chunk, heads strided in groups of 65 (64 v + ones)
            v_sb = [acp.tile([128, HPC * (D + 1)], BF16, name=f"v{ct}",
                             tag=f"v{ct}") for ct in range(CT)]

            with tc.tile_pool(name="pps", bufs=4, space="PSUM") as pps:
                # K and Q projections -> feature-major + per-partition bias
                for t in range(DT):
                    for qb in range(lc // 512):
                        ps = pps.tile([128, 512], F32, name="kq_ps", tag="ps")
                        for a in range(NT):
                            nc.tensor.matmul(
                                ps[:],
                                wk_sb[a][:, t * 128:(t + 1) * 128],
                                cTi[a][:, qb * 512:(qb + 1) * 512],
                                start=(a == 0), stop=(a == NT - 1))
                        nc.vector.tensor_scalar_add(
                            kT_sb[t][:, qb * 512:(qb + 1) * 512], ps[:],
                            bk_sb[t][:])
                for t in range(DT):
                    for qb in range(lq // 512):
                        ps = pps.tile([128, 512], F32, name="kq_ps", tag="ps")
                        for a in range(NT):
                            nc.tensor.matmul(
                                ps[:],
                                wq_sb[a][:, t * 128:(t + 1) * 128],
                                qTi[a][:, qb * 512:(qb + 1) * 512],
                                start=(a == 0), stop=(a == NT - 1))
                        nc.vector.tensor_scalar_add(
                            qT_sb[t][:, qb * 512:(qb + 1) * 512], ps[:],
                            bq_sb[t][:])
                # V projection (token-major) + bias via ones-row matmul
                for ct in range(CT):
                    ps = pps.tile([128, HD], F32, name="v_ps", tag="ps")
                    for a in range(NT):
                        nc.tensor.matmul(
                            ps[:],
                            cTi[a][:, ct * 128:(ct + 1) * 128],
                            wv_sb[a][:],
                            start=(a == 0), stop=False)
                    nc.tensor.matmul(
                        ps[:],
                        ones_c[:, ct * 128:(ct + 1) * 128],
                        wvb_sb[:],
                        start=False, stop=True)
                    vg = v_sb[ct].rearrange("p (g x) -> p g x", x=D + 1)
                    nc.vector.tensor_copy(vg[:, :, 0:D],
                                          ps.rearrange("p (g x) -> p g x", x=D))
                    nc.vector.memset(vg[:, :, D:D + 1], 1.0)

            # ---- attention + output projection ----
            attn_sb = [acp.tile([128, lq], BF16, name=f"attn{t}",
                                tag=f"attn{t}") for t in range(DT)]

            with tc.tile_pool(name="st_ps", bufs=2, space="PSUM") as stp, \
                 tc.tile_pool(name="av_ps", bufs=1, space="PSUM") as avp, \
                 tc.tile_pool(name="op_ps", bufs=2, space="PSUM") as opp:
                for qe in range(NQE):
                    for h in range(HPC):
                        t, r0 = h // 2, (h % 2) * 64
                        # scores (transposed) + exp -> PT tiles
                        pts = []
                        for ct in range(CT):
                            st = stp.tile([128, QE], F32, name="st", tag="st")
                            for s in range(NS):
                                nc.tensor.matmul(
                                    st[:, s * 512:(s + 1) * 512],
                                    kT_sb[t][r0:r0 + 64, ct * 128:(ct + 1) * 128],
                                    qT_sb[t][r0:r0 + 64,
                                             qe * QE + s * 512:
                                             qe * QE + (s + 1) * 512],
                                    start=True, stop=True)
                            pt = ptp.tile([128, QE], BF16, name="pt", tag="pt")
                            nc.scalar.activation(
                                pt[:], st[:],
                                mybir.ActivationFunctionType.Exp, scale=SCALE)
                            pts.append(pt)
                        # attention @ V' (row 64 of result = Z)
                        av = avp.tile([65, QE], F32, name="av", tag="av")
                        for ct in range(CT):
                            for s in range(NS):
                                nc.tensor.matmul(
                                    av[:, s * 512:(s + 1) * 512],
                                    v_sb[ct][:, h * (D + 1):(h + 1) * (D + 1)],
                                    pts[ct][:, s * 512:(s + 1) * 512],
                                    start=(ct == 0), stop=(ct == CT - 1))
                        # normalize: 1/Z, broadcast over 64 partitions, mult
                        for s in range(NS):
                            q0 = qe * QE + s * 512
                            rz = nrm.tile([1, 512], F32, name="rz", tag="rz")
                            nc.vector.reciprocal(
                                rz[:], av[64:65, s * 512:(s + 1) * 512])
                            rb = nrm.tile([64, 512], F32, name="rb", tag="rb")
                            nc.gpsimd.partition_broadcast(rb[:], rz[:])
                            if h % 2 == 0:
                                nc.vector.tensor_mul(
                                    attn_sb[t][0:64, q0:q0 + 512],
                                    av[0:64, s * 512:(s + 1) * 512], rb[:])
                            else:
                                at = nrm.tile([64, 512], BF16, name="at",
                                              tag="at")
                                nc.vector.tensor_mul(
                                    at[:], av[0:64, s * 512:(s + 1) * 512],
                                    rb[:])
                                nc.sync.dma_start(
                                    attn_sb[t][64:128, q0:q0 + 512], at[:])
                    # output projection for this q range (partial over heads)
                    for qt in range(qe * QE // 128, (qe + 1) * QE // 128):
                        for nh in range(MODEL // 512):
                            ps = opp.tile([128, 512], F32, name="o_ps",
                                          tag="o_ps")
                            for t in range(DT):
                                nc.tensor.matmul(
                                    ps[:],
                                    attn_sb[t][:, qt * 128:(qt + 1) * 128],
                                    wo_sb[t][:, nh * 512:(nh + 1) * 512],
                                    start=(t == 0), stop=(t == DT - 1))
                            ot = osb.tile([128, 512], F32, name="ot", tag="ot")
                            nc.vector.tensor_copy(ot[:], ps[:])
                            nc.sync.dma_start(
                                out_d[qt * 128:(qt + 1) * 128,
                                      nh * 512:(nh + 1) * 512], ot[:])
    nc.compile()
    return nc


def _prep_inputs(query, context, Wq, bq, Wkv, bkv, Wout, lq, lc):
    """Host-side shard/cast/pack. Returns in_maps for 8 cores."""
    Wkv_r = np.asarray(Wkv, np.float32).reshape(MODEL, H, D, 2)
    bkv_r = np.asarray(bkv, np.float32).reshape(H, D, 2)
    Wq = np.asarray(Wq, np.float32)
    bq = np.asarray(bq, np.float32)
    Wout = np.asarray(Wout, np.float32)

    qT = [np.ascontiguousarray(np.asarray(query[b], np.float32).T).astype(NPBF16)
          for b in range(B)]
    cT = [np.ascontiguousarray(np.asarray(context[b], np.float32).T).astype(NPBF16)
          for b in range(B)]

    grp = []
    for g in range(GROUPS):
        hs = slice(g * HPC, (g + 1) * HPC)
        wq_c = Wq[:, g * HD:(g + 1) * HD].astype(NPBF16)
        wk_c = np.ascontiguousarray(
            Wkv_r[:, hs, :, 0].reshape(MODEL, HD)).astype(NPBF16)
        wv_c = Wkv_r[:, hs, :, 1].reshape(MODEL, HD)
        bv_c = bkv_r[hs, :, 1].reshape(1, HD)
        wv_c = np.concatenate([wv_c, bv_c], 0).astype(NPBF16)
        wo_c = np.ascontiguousarray(
            Wout[g * HD:(g + 1) * HD, :]).astype(NPBF16)
        bq_c = np.ascontiguousarray(
            bq[g * HD:(g + 1) * HD].reshape(HD, 1), dtype=np.float32)
        bk_c = np.ascontiguousarray(
            bkv_r[hs, :, 0].reshape(HD, 1), dtype=np.float32)
        grp.append((wq_c, wk_c, wv_c, wo_c, bq_c, bk_c))

    in_maps = []
    for c in range(N_CORES):
        b, g = c // GROUPS, c % GROUPS
        wq_c, wk_c, wv_c, wo_c, bq_c, bk_c = grp[g]
        in_maps.append({
            "qT": qT[b], "cT": cT[b],
            "wq": wq_c, "wk": wk_c, "wv": wv_c, "wo": wo_c,
            "bq": bq_c, "bk": bk_c,
        })
    return in_maps


class Runner:
    """Cached-jit PJRT executor for the SPMD bass kernel (axon path).

    Mirrors bass2jax.run_bass_via_pjrt's multi-core branch, but builds the
    jitted sharded callable once so repeated calls reuse the compiled
    executable (for steady-state timing) and inputs can be pre-placed on
    device.
    """

    def __init__(self, nc, n_cores=N_CORES):
        import jax
        from jax.sharding import Mesh, PartitionSpec, NamedSharding
        from jax.experimental.shard_map import shard_map

        bass2jax.install_neuronx_cc_hook()
        assert nc.dbg_addr is None
        part_name = (nc.partition_id_tensor.name
                     if nc.partition_id_tensor else None)

        in_names, out_names, out_avals, zero_outs = [], [], [], []
        for alloc in nc.m.functions[0].allocations:
            if not isinstance(alloc, mybir.MemoryLocationSet):
                continue
            name = alloc.memorylocations[0].name
            if alloc.kind == "ExternalInput":
                if name != part_name:
                    in_names.append(name)
            elif alloc.kind == "ExternalOutput":
                shape = tuple(alloc.tensor_shape)
                dtype = mybir.dt.np(alloc.dtype)
                out_names.append(name)
                out_avals.append(jax.core.ShapedArray(shape, dtype))
                zero_outs.append(np.zeros(shape, dtype))
        self.n_params = len(in_names)
        self.in_names = list(in_names)
        self.out_names = out_names
        self.out_avals = out_avals
        self.zero_outs = zero_outs
        all_names = tuple(
            in_names + out_names + ([part_name] if part_name else []))

        def _body(*args):
            operands = list(args)
            if part_name is not None:
                operands.append(bass2jax.partition_id_tensor())
            outs = bass2jax._bass_exec_p.bind(
                *operands,
                out_avals=tuple(out_avals),
                in_names=all_names,
                out_names=tuple(out_names),
                lowering_input_output_aliases=(),
                sim_require_finite=True,
                sim_require_nnan=True,
                nc=nc,
            )
            return tuple(outs)

        devices = jax.devices()[:n_cores]
        assert len(devices) == n_cores
        self.n_cores = n_cores
        self.mesh = Mesh(np.asarray(devices), ("core",))
        self.sharding = NamedSharding(self.mesh, PartitionSpec("core"))
        n_args = self.n_params + len(out_names)
        self.fn = jax.jit(
            shard_map(_body, mesh=self.mesh,
                      in_specs=(PartitionSpec("core"),) * n_args,
                      out_specs=(PartitionSpec("core"),) * len(out_names),
                      check_rep=False),
            keep_unused=True,
        )
        self._jax = jax

    def put(self, in_maps):
        """Concatenate per-core inputs on axis 0 and place on devices."""
        jax = self._jax
        args = []
        for i, name in enumerate(self.in_names):
            arr = np.concatenate(
                [np.asarray(m[name]) for m in in_maps], axis=0)
            args.append(jax.device_put(arr, self.sharding))
        for z in self.zero_outs:
            zz = np.zeros((self.n_cores * z.shape[0], *z.shape[1:]), z.dtype)
            args.append(jax.device_put(zz, self.sharding))
        return args

    def call(self, dev_args):
        outs = self.fn(*dev_args)
        self._jax.block_until_ready(outs)
        return outs

    def gather(self, outs):
        """outs -> list (per core) of {name: np.ndarray}."""
        res = []
        for c in range(self.n_cores):
            d = {}
            for i, name in enumerate(self.out_names):
                full = np.asarray(outs[i])
                d[name] = full.reshape(
                    self.n_cores, *self.out_avals[i].shape)[c]
            res.append(d)
        return res


_CACHE = {}


def _get_runner(lq, lc):
    key = (lq, lc)
    if key not in _CACHE:
        _CACHE[key] = Runner(build_nc(lq, lc))
    return _CACHE[key]


def _reduce_out(results, bout, lq):
    bout = np.asarray(bout, np.float32)
    out = np.empty((B, lq, MODEL), np.float32)
    for b in range(B):
        acc = results[b * GROUPS]["outp"].copy()
        for g in range(1, GROUPS):
            acc += results[b * GROUPS + g]["outp"]
        out[b] = acc + bout
    return out


def run(query, context, Wq, bq, Wkv, bkv, Wout, bout):
    lq, lc = query.shape[1], context.shape[1]
    runner = _get_runner(lq, lc)
    in_maps = _prep_inputs(query, context, Wq, bq, Wkv, bkv, Wout, lq, lc)
    dev_args = runner.put(in_maps)
    outs = runner.call(dev_args)
    results = runner.gather(outs)
    return _reduce_out(results, bout, lq), runner, dev_args


def kernel(query, context, Wq, bq, Wkv, bkv, Wout, bout):
    out, _, _ = run(query, context, Wq, bq, Wkv, bkv, Wout, bout)
    return out


# revision 15
# speedup vs baseline: 253.9928x; 253.9928x over previous
"""CrossAttention Trainium2 kernel.

Full inputs in, full output out. Sharding: data-parallel over batch (B=2),
tensor-parallel over heads (16 heads -> 4 groups of 4), 8 cores total.
Each core computes attention for its (batch, 4-head group) and a partial
output projection; the host sums the 4 partials per batch (Megatron-style
row-parallel Wout reduce) and adds bout.

Device-side dataflow is feature-major end to end:
  v[c,d+1]   = context @ Wv_slice (+ones col per head for softmax denom)
  qT[d,q]    = (Wq_slice).T @ query.T      (lhsT=Wq slice, rhs=queryT)
  kT[d,c]    = (Wk_slice).T @ context.T
  ST[c,q]    = kT.T-slice @ qT-slice       (scores, transposed; K=64 row-
               groups alternate between the two heads of a pair -> the PE
               runs them concurrently)
  PT[c,q]    = exp(SCALE * ST)             (ScalarE, scale fused)
  att[d+1,q] = v'.T @ PT                   (row 64 = sum_c PT = Z)
  att_n      = att[0:64] * broadcast(1/Z)  (DVE + gpsimd partition_broadcast)
  out[q,n]   = att_n-chunks.T @ Wout-chunks  (token-major, partial)
"""

import numpy as np
import ml_dtypes

import concourse.bass as bass
import concourse.tile as tile
from concourse import bacc, mybir
from concourse import bass2jax

BF16 = mybir.dt.bfloat16
F32 = mybir.dt.float32
NPBF16 = ml_dtypes.bfloat16

B, MODEL, H, D = 2, 1024, 16, 64
LQ, LC = 2048, 2048
N_CORES = 8
GROUPS = 4            # head groups = cores per batch
HPC = H // GROUPS     # heads per core
HD = HPC * D          # 256 head-dims per core
SCALE = D ** -0.5
EXPF = mybir.ActivationFunctionType.Exp


def build_nc(lq=LQ, lc=LC, reps=1):
    NT = MODEL // 128         # m-contraction chunks
    DT = HD // 128            # d-tiles (128 rows each) for qT/kT
    CT = lc // 128            # context chunks
    QE = 1024 if lq % 1024 == 0 else 512   # exp block (free dim of ST)
    NQE = lq // QE
    NS = QE // 512            # 512-sub-blocks per exp block

    nc = bacc.Bacc(trn_type="TRN2", target_bir_lowering=False, debug=False,
                   num_devices=N_CORES)

    qT_d = nc.dram_tensor("qT", [MODEL, lq], BF16, kind="ExternalInput").ap()
    cT_d = nc.dram_tensor("cT", [MODEL, lc], BF16, kind="ExternalInput").ap()
    wq_d = nc.dram_tensor("wq", [MODEL, HD], BF16, kind="ExternalInput").ap()
    wk_d = nc.dram_tensor("wk", [MODEL, HD], BF16, kind="ExternalInput").ap()
    wv_d = nc.dram_tensor("wv", [MODEL + 1, HD], BF16, kind="ExternalInput").ap()
    wo_d = nc.dram_tensor("wo", [HD, MODEL], BF16, kind="ExternalInput").ap()
    bq_d = nc.dram_tensor("bq", [HD, 1], F32, kind="ExternalInput").ap()
    bk_d = nc.dram_tensor("bk", [HD, 1], F32, kind="ExternalInput").ap()
    out_d = nc.dram_tensor("outp", [lq, MODEL], F32, kind="ExternalOutput").ap()

    with tile.TileContext(nc) as tc:
        with tc.tile_pool(name="w", bufs=1) as wp, \
             tc.tile_pool(name="acts", bufs=1) as acp, \
             tc.tile_pool(name="ptp", bufs=12) as ptp, \
             tc.tile_pool(name="nrm", bufs=4) as nrm, \
             tc.tile_pool(name="osb", bufs=4) as osb:

            # ---- load weights / biases ----
            wq_sb = [wp.tile([128, HD], BF16, name=f"wq{a}", tag=f"wq{a}")
                     for a in range(NT)]
            wk_sb = [wp.tile([128, HD], BF16, name=f"wk{a}", tag=f"wk{a}")
                     for a in range(NT)]
            wv_sb = [wp.tile([128, HD], BF16, name=f"wv{a}", tag=f"wv{a}")
                     for a in range(NT)]
            wvb_sb = wp.tile([1, HD], BF16, name="wvb", tag="wvb")
            wo_sb = [wp.tile([128, MODEL], BF16, name=f"wo{t}", tag=f"wo{t}")
                     for t in range(DT)]
            bq_sb = [wp.tile([128, 1], F32, name=f"bq{t}", tag=f"bq{t}")
                     for t in range(DT)]
            bk_sb = [wp.tile([128, 1], F32, name=f"bk{t}", tag=f"bk{t}")
                     for t in range(DT)]
            for a in range(NT):
                nc.sync.dma_start(wv_sb[a][:], wv_d[a * 128:(a + 1) * 128, :])
                nc.sync.dma_start(wk_sb[a][:], wk_d[a * 128:(a + 1) * 128, :])
                nc.sync.dma_start(wq_sb[a][:], wq_d[a * 128:(a + 1) * 128, :])
            nc.sync.dma_start(wvb_sb[:], wv_d[MODEL:MODEL + 1, :])
            for t in range(DT):
                nc.sync.dma_start(wo_sb[t][:], wo_d[t * 128:(t + 1) * 128, :])
                nc.sync.dma_start(bq_sb[t][:], bq_d[t * 128:(t + 1) * 128, :])
                nc.sync.dma_start(bk_sb[t][:], bk_d[t * 128:(t + 1) * 128, :])

            # ---- load transposed activations ----
            qTi = [acp.tile([128, lq], BF16, name=f"qTi{a}", tag=f"qTi{a}")
                   for a in range(NT)]
            cTi = [acp.tile([128, lc], BF16, name=f"cTi{a}", tag=f"cTi{a}")
                   for a in range(NT)]
            for a in range(NT):
                nc.sync.dma_start(cTi[a][:], cT_d[a * 128:(a + 1) * 128, :])
                nc.sync.dma_start(qTi[a][:], qT_d[a * 128:(a + 1) * 128, :])
            ones_c = acp.tile([1, lc], BF16, name="ones_c", tag="ones_c")
            nc.vector.memset(ones_c[:], 1.0)

            kT_sb = [acp.tile([128, lc], BF16, name=f"kT{t}", tag=f"kT{t}")
                     for t in range(DT)]
            qT_sb = [acp.tile([128, lq], BF16, name=f"qT{t}", tag=f"qT{t}")
                     for t in range(DT)]
            v_sb = [acp.tile([128, HPC * (D + 1)], BF16, name=f"v{ct}",
                             tag=f"v{ct}") for ct in range(CT)]
            attn_sb = [acp.tile([128, lq], BF16, name=f"attn{t}",
                                tag=f"attn{t}") for t in range(DT)]

            def emit_projections():
                with tc.tile_pool(name="pps", bufs=4, space="PSUM") as pps:
                    # K/Q for the first head pair first, so attention's score
                    # stream (and ScalarE exp) can start as early as possible;
                    # V and the second d-tile follow.
                    order = [("kq", 0), ("v", None)] + \
                            [("kq", t) for t in range(1, DT)]
                    for kind, t in order:
                        if kind == "v":
                            for ct in range(CT):
                                ps = pps.tile([128, HD], F32, name="v_ps",
                                              tag="ps")
                                for a in range(NT):
                                    nc.tensor.matmul(
                                        ps[:],
                                        cTi[a][:, ct * 128:(ct + 1) * 128],
                                        wv_sb[a][:], start=(a == 0),
                                        stop=False)
                                nc.tensor.matmul(
                                    ps[:], ones_c[:, ct * 128:(ct + 1) * 128],
                                    wvb_sb[:], start=False, stop=True)
                                vg = v_sb[ct].rearrange("p (g x) -> p g x",
                                                        x=D + 1)
                                nc.vector.tensor_copy(
                                    vg[:, :, 0:D],
                                    ps.rearrange("p (g x) -> p g x", x=D))
                                nc.vector.memset(vg[:, :, D:D + 1], 1.0)
                            continue
                        for qb in range(lc // 512):
                            ps = pps.tile([128, 512], F32, name="k_ps",
                                          tag="ps")
                            for a in range(NT):
                                nc.tensor.matmul(
                                    ps[:],
                                    wk_sb[a][:, t * 128:(t + 1) * 128],
                                    cTi[a][:, qb * 512:(qb + 1) * 512],
                                    start=(a == 0), stop=(a == NT - 1))
                            nc.vector.tensor_scalar_add(
                                kT_sb[t][:, qb * 512:(qb + 1) * 512], ps[:],
                                bk_sb[t][:])
                        for qb in range(lq // 512):
                            ps = pps.tile([128, 512], F32, name="q_ps",
                                          tag="ps")
                            for a in range(NT):
                                nc.tensor.matmul(
                                    ps[:],
                                    wq_sb[a][:, t * 128:(t + 1) * 128],
                                    qTi[a][:, qb * 512:(qb + 1) * 512],
                                    start=(a == 0), stop=(a == NT - 1))
                            nc.vector.tensor_scalar_add(
                                qT_sb[t][:, qb * 512:(qb + 1) * 512], ps[:],
                                bq_sb[t][:])

            def emit_attention():
                with tc.tile_pool(name="st_ps", bufs=2, space="PSUM") as stp, \
                     tc.tile_pool(name="av_ps", bufs=2, space="PSUM") as avp:
                    for qe in range(NQE):
                        for hp in range(HPC // 2):
                            avs = [avp.tile([65, QE], F32, name=f"av{e}",
                                            tag="av") for e in (0, 1)]
                            for ct in range(CT):
                                sts, pts = [], []
                                for e in (0, 1):
                                    st = stp.tile([128, QE], F32,
                                                  name=f"st{e}", tag="st")
                                    sts.append(st)
                                # alternate row groups (even head = rows
                                # 0:64, odd = 64:128) so the PE overlaps the
                                # K=64 matmuls of the two heads.
                                for s in range(NS):
                                    for e in (0, 1):
                                        r0 = e * 64
                                        nc.tensor.matmul(
                                            sts[e][:, s * 512:(s + 1) * 512],
                                            kT_sb[hp][r0:r0 + 64,
                                                      ct * 128:(ct + 1) * 128],
                                            qT_sb[hp][r0:r0 + 64,
                                                      qe * QE + s * 512:
                                                      qe * QE + (s + 1) * 512],
                                            start=True, stop=True)
                                for e in (0, 1):
                                    pt = ptp.tile([128, QE], BF16, name="pt",
                                                  tag="pt")
                                    nc.scalar.activation(pt[:], sts[e][:],
                                                         EXPF, scale=SCALE)
                                    pts.append(pt)
                                for e in (0, 1):
                                    h = 2 * hp + e
                                    for s in range(NS):
                                        nc.tensor.matmul(
                                            avs[e][:, s * 512:(s + 1) * 512],
                                            v_sb[ct][:, h * (D + 1):
                                                     (h + 1) * (D + 1)],
                                            pts[e][:, s * 512:(s + 1) * 512],
                                            start=(ct == 0),
                                            stop=(ct == CT - 1))
                            # normalize: 1/Z broadcast over partitions
                            for e in (0, 1):
                                for s in range(NS):
                                    q0 = qe * QE + s * 512
                                    rz = nrm.tile([1, 512], F32, name="rz",
                                                  tag="rz")
                                    nc.vector.reciprocal(
                                        rz[:],
                                        avs[e][64:65, s * 512:(s + 1) * 512])
                                    rb = nrm.tile([64, 512], F32, name="rb",
                                                  tag="rb")
                                    nc.gpsimd.partition_broadcast(rb[:], rz[:])
                                    if e == 0:
                                        nc.vector.tensor_mul(
                                            attn_sb[hp][0:64, q0:q0 + 512],
                                            avs[e][0:64, s * 512:(s + 1) * 512],
                                            rb[:])
                                    else:
                                        at = nrm.tile([64, 512], BF16,
                                                      name="at", tag="at")
                                        nc.vector.tensor_mul(
                                            at[:],
                                            avs[e][0:64, s * 512:(s + 1) * 512],
                                            rb[:])
                                        nc.sync.dma_start(
                                            attn_sb[hp][64:128, q0:q0 + 512],
                                            at[:])

            def emit_outproj():
                with tc.tile_pool(name="op_ps", bufs=4, space="PSUM") as opp:
                    for qt in range(lq // 128):
                        for nh in range(MODEL // 512):
                            ps = opp.tile([128, 512], F32, name="o_ps",
                                          tag="o_ps")
                            for t in range(DT):
                                nc.tensor.matmul(
                                    ps[:],
                                    attn_sb[t][:, qt * 128:(qt + 1) * 128],
                                    wo_sb[t][:, nh * 512:(nh + 1) * 512],
                                    start=(t == 0), stop=(t == DT - 1))
                            ot = osb.tile([128, 512], F32, name="ot", tag="ot")
                            nc.vector.tensor_copy(ot[:], ps[:])
                            nc.sync.dma_start(
                                out_d[qt * 128:(qt + 1) * 128,
                                      nh * 512:(nh + 1) * 512], ot[:])

            for _ in range(reps):
                emit_projections()
                emit_attention()
                emit_outproj()

    nc.compile()
    return nc


def _prep_inputs(query, context, Wq, bq, Wkv, bkv, Wout, lq, lc):
    """Host-side shard/cast/pack. Returns in_maps for 8 cores."""
    Wkv_r = np.asarray(Wkv, np.float32).reshape(MODEL, H, D, 2)
    bkv_r = np.asarray(bkv, np.float32).reshape(H, D, 2)
    Wq = np.asarray(Wq, np.float32)
    bq = np.asarray(bq, np.float32)
    Wout = np.asarray(Wout, np.float32)

    qT = [np.ascontiguousarray(np.asarray(query[b], np.float32).T).astype(NPBF16)
          for b in range(B)]
    cT = [np.ascontiguousarray(np.asarray(context[b], np.float32).T).astype(NPBF16)
          for b in range(B)]

    grp = []
    for g in range(GROUPS):
        hs = slice(g * HPC, (g + 1) * HPC)
        wq_c = Wq[:, g * HD:(g + 1) * HD].astype(NPBF16)
        wk_c = np.ascontiguousarray(
            Wkv_r[:, hs, :, 0].reshape(MODEL, HD)).astype(NPBF16)
        wv_c = Wkv_r[:, hs, :, 1].reshape(MODEL, HD)
        bv_c = bkv_r[hs, :, 1].reshape(1, HD)
        wv_c = np.concatenate([wv_c, bv_c], 0).astype(NPBF16)
        wo_c = np.ascontiguousarray(
            Wout[g * HD:(g + 1) * HD, :]).astype(NPBF16)
        bq_c = np.ascontiguousarray(
            bq[g * HD:(g + 1) * HD].reshape(HD, 1), dtype=np.float32)
        bk_c = np.ascontiguousarray(
            bkv_r[hs, :, 0].reshape(HD, 1), dtype=np.float32)
        grp.append((wq_c, wk_c, wv_c, wo_c, bq_c, bk_c))

    in_maps = []
    for c in range(N_CORES):
        b, g = c // GROUPS, c % GROUPS
        wq_c, wk_c, wv_c, wo_c, bq_c, bk_c = grp[g]
        in_maps.append({
            "qT": qT[b], "cT": cT[b],
            "wq": wq_c, "wk": wk_c, "wv": wv_c, "wo": wo_c,
            "bq": bq_c, "bk": bk_c,
        })
    return in_maps


class Runner:
    """Cached-jit PJRT executor for the SPMD bass kernel (axon path).

    Mirrors bass2jax.run_bass_via_pjrt's multi-core branch, but builds the
    jitted sharded callable once so repeated calls reuse the compiled
    executable (for steady-state timing) and inputs can be pre-placed on
    device.
    """

    def __init__(self, nc, n_cores=N_CORES):
        import jax
        from jax.sharding import Mesh, PartitionSpec, NamedSharding
        from jax.experimental.shard_map import shard_map

        bass2jax.install_neuronx_cc_hook()
        assert nc.dbg_addr is None
        part_name = (nc.partition_id_tensor.name
                     if nc.partition_id_tensor else None)

        in_names, out_names, out_avals, zero_outs = [], [], [], []
        for alloc in nc.m.functions[0].allocations:
            if not isinstance(alloc, mybir.MemoryLocationSet):
                continue
            name = alloc.memorylocations[0].name
            if alloc.kind == "ExternalInput":
                if name != part_name:
                    in_names.append(name)
            elif alloc.kind == "ExternalOutput":
                shape = tuple(alloc.tensor_shape)
                dtype = mybir.dt.np(alloc.dtype)
                out_names.append(name)
                out_avals.append(jax.core.ShapedArray(shape, dtype))
                zero_outs.append(np.zeros(shape, dtype))
        self.n_params = len(in_names)
        self.in_names = list(in_names)
        self.out_names = out_names
        self.out_avals = out_avals
        self.zero_outs = zero_outs
        all_names = tuple(
            in_names + out_names + ([part_name] if part_name else []))

        def _body(*args):
            operands = list(args)
            if part_name is not None:
                operands.append(bass2jax.partition_id_tensor())
            outs = bass2jax._bass_exec_p.bind(
                *operands,
                out_avals=tuple(out_avals),
                in_names=all_names,
                out_names=tuple(out_names),
                lowering_input_output_aliases=(),
                sim_require_finite=True,
                sim_require_nnan=True,
                nc=nc,
            )
            return tuple(outs)

        devices = jax.devices()[:n_cores]
        assert len(devices) == n_cores
        self.n_cores = n_cores
        self.mesh = Mesh(np.asarray(devices), ("core",))
        self.sharding = NamedSharding(self.mesh, PartitionSpec("core"))
        n_args = self.n_params + len(out_names)
        self.fn = jax.jit(
            shard_map(_body, mesh=self.mesh,
                      in_specs=(PartitionSpec("core"),) * n_args,
                      out_specs=(PartitionSpec("core"),) * len(out_names),
                      check_rep=False),
            keep_unused=True,
        )
        self._jax = jax

    def put(self, in_maps):
        """Concatenate per-core inputs on axis 0 and place on devices."""
        jax = self._jax
        args = []
        for name in self.in_names:
            arr = np.concatenate(
                [np.asarray(m[name]) for m in in_maps], axis=0)
            args.append(jax.device_put(arr, self.sharding))
        for z in self.zero_outs:
            zz = np.zeros((self.n_cores * z.shape[0], *z.shape[1:]), z.dtype)
            args.append(jax.device_put(zz, self.sharding))
        return args

    def call(self, dev_args):
        outs = self.fn(*dev_args)
        self._jax.block_until_ready(outs)
        return outs

    def gather(self, outs):
        """outs -> list (per core) of {name: np.ndarray}."""
        res = []
        for c in range(self.n_cores):
            d = {}
            for i, name in enumerate(self.out_names):
                full = np.asarray(outs[i])
                d[name] = full.reshape(
                    self.n_cores, *self.out_avals[i].shape)[c]
            res.append(d)
        return res


_CACHE = {}


def _get_runner(lq, lc):
    key = (lq, lc)
    if key not in _CACHE:
        _CACHE[key] = Runner(build_nc(lq, lc))
    return _CACHE[key]


def _reduce_out(results, bout, lq):
    bout = np.asarray(bout, np.float32)
    out = np.empty((B, lq, MODEL), np.float32)
    for b in range(B):
        acc = results[b * GROUPS]["outp"].copy()
        for g in range(1, GROUPS):
            acc += results[b * GROUPS + g]["outp"]
        out[b] = acc + bout
    return out


def run(query, context, Wq, bq, Wkv, bkv, Wout, bout):
    lq, lc = query.shape[1], context.shape[1]
    runner = _get_runner(lq, lc)
    in_maps = _prep_inputs(query, context, Wq, bq, Wkv, bkv, Wout, lq, lc)
    dev_args = runner.put(in_maps)
    outs = runner.call(dev_args)
    results = runner.gather(outs)
    return _reduce_out(results, bout, lq), runner, dev_args


def kernel(query, context, Wq, bq, Wkv, bkv, Wout, bout):
    out, _, _ = run(query, context, Wq, bq, Wkv, bkv, Wout, bout)
    return out
